# revision 10
# baseline (speedup 1.0000x reference)
"""GQA kernel for Trainium2, sharded over 8 NeuronCores.

Sharding: data-parallel over batch (2) x tensor-parallel over kv_heads (4).
Core c = b*4 + h computes the full attention output partial
    Y_bh = softmax(causal((Q_b @ Wq_eff_h) @ (K_b @ Wk_h)^T / sqrt(dk))) @ (V_b @ Wv_h) @ Wo_h
and the host sums the 4 head partials per batch (the "all-reduce after Wo").

The GQA group-sum-before-softmax quirk folds into the weights:
    scores_h = sum_g (Q Wq_{g,h}) (K Wk_h)^T = (Q [sum_g Wq_{g,h}]) (K Wk_h)^T
so Wq_eff_h = sum_g Wq[:, (g*KV+h)*dk : ...] and each core runs standard attention.

Device schedule (single in-order pass, DMA-wire is the binding resource):
  1. k-projection, q-projection (activation rows streamed as 512KB chunks)
  2. v-projection DMA/matmuls interleaved with ALL attention score work
     (S^T tiles -> exp -> causal mask -> ones-matmul row sums); the exp'd
     P^T tiles stay resident in SBUF (40KB/partition) so the PV matmuls
     can wait for v without stalling the scores.
  3. v transposes (PE), then per query-chunk: PV accumulation, softmax
     normalization folded into the O^T eviction, Y = O @ Wo, fp16 out.

Layouts (SBUF partition dim first): qT/kT/vT (dk=128, L) fp16; S^T tiles
(Lk_t=128, Lq=512) fp32 psum; row sums via ones-matmul (result replicated
across partitions == the free-dim broadcast needed to normalize O^T).
"""
import sys
sys.path.insert(0, '/opt/trn_rl_repo')
import math
import numpy as np

import concourse.bass as bass
import concourse.mybir as mybir
import concourse.tile as tile
from concourse import bacc
from concourse import bass_utils
from concourse.masks import make_identity

FP32 = mybir.dt.float32
FP16 = mybir.dt.float16

B, L, D = 2, 2048, 2048
Q_HEADS, KV_HEADS, DK, DV = 16, 4, 128, 128
GROUPS = Q_HEADS // KV_HEADS
P = 128
CH = 512                 # Lq chunk width
NJ = L // CH             # 4 query chunks
NDC = D // P             # 16 contraction chunks
NLK = L // P             # 16 key tiles
SCALE = 1.0 / math.sqrt(DK)
EBIAS = -8.0 * math.log(2.0)   # exp output scaled by 2^-8; cancels in softmax
YDT = FP16               # partial-output dtype (host accumulates in fp32)
YNP = np.float16

# flattened score work items (j, c), j-major so rrep accumulators stay serial
SCORE_ITEMS = [(j, c) for j in range(NJ) for c in range(4 * j + 4)]
ET_OFF = {}
_off = 0
for _j, _c in SCORE_ITEMS:
    ET_OFF[(_j, _c)] = _off
    _off += CH
ET_W = _off              # 40 * 512 fp16 = 40KB/partition


def _build():
    nc = bacc.Bacc(trn_type="TRN2")
    qt_d = nc.dram_tensor("qt", (D, L), FP16, kind="ExternalInput")
    kt_d = nc.dram_tensor("kt", (D, L), FP16, kind="ExternalInput")
    vt_d = nc.dram_tensor("vt", (D, L), FP16, kind="ExternalInput")
    # weights pre-packed on host to the SBUF image: (128, NDC*dk)
    wq_d = nc.dram_tensor("wq", (P, NDC * DK), FP16, kind="ExternalInput")
    wk_d = nc.dram_tensor("wk", (P, NDC * DK), FP16, kind="ExternalInput")
    wv_d = nc.dram_tensor("wv", (P, NDC * DV), FP16, kind="ExternalInput")
    wo_d = nc.dram_tensor("wo", (DV, D), FP16, kind="ExternalInput")
    mask_d = nc.dram_tensor("mask", (P, NJ * CH), FP16, kind="ExternalInput")
    y_d = nc.dram_tensor("y", (L, D), YDT, kind="ExternalOutput")

    with tile.TileContext(nc) as tc:
        with (
            tc.tile_pool(name="const", bufs=1) as const,
            tc.tile_pool(name="wpool", bufs=1) as wpool,
            tc.tile_pool(name="xs", bufs=6) as xs,
            tc.tile_pool(name="proj", bufs=1) as proj,
            tc.tile_pool(name="rinvp", bufs=2) as rinvp,
            tc.tile_pool(name="ev", bufs=4) as ev_pool,
            tc.tile_pool(name="ps", bufs=7, space="PSUM") as ps,
        ):
            ident = const.tile([P, P], FP16)
            make_identity(nc, ident[:])
            ones = const.tile([P, P], FP16)
            nc.vector.memset(ones[:], 1.0)
            ones2 = const.tile([P, 256], FP16)
            nc.vector.memset(ones2[:], 1.0)
            ebias = const.tile([P, 1], FP32)
            nc.vector.memset(ebias[:], EBIAS)

            kT = proj.tile([P, L], FP16, tag="kT")
            qT = proj.tile([P, L], FP16, tag="qT")
            vT = proj.tile([P, L], FP16, tag="vT")
            v_nat = proj.tile([P, L], FP16, tag="v_nat")
            oT = proj.tile([P, L], FP16, tag="oT")
            et_all = proj.tile([P, ET_W], FP16, tag="et_all")
            rinv_all = proj.tile([P, NJ * CH], FP32, tag="rinv_all")

            w_sbs = {}
            maskt = const.tile([P, NJ * CH], FP16)

            def load_w(name, wd):
                w_sb = wpool.tile([P, NDC * DK], FP16, tag=name, name=name)
                nc.scalar.dma_start(w_sb[:], wd[:])
                w_sbs[name] = w_sb

            warm = ps.tile([P, 256], FP32, tag="warm", bufs=1, name="warm")

            def project(xt_dram, wname, dst, fill=False):
                w_sb = w_sbs[wname]
                accs = [ps.tile([P, CH], FP32, tag="ps", name=f"acc{j}")
                        for j in range(NJ)]
                for dc in range(NDC):
                    xt = xs.tile([P, L], FP16, tag="xt", name="xt")
                    nc.sync.dma_start(xt[:], xt_dram[dc * P:(dc + 1) * P, :])
                    for j in range(NJ):
                        nc.tensor.matmul(
                            accs[j][:], w_sb[:, dc * P:dc * P + P],
                            xt[:, j * CH:(j + 1) * CH],
                            start=(dc == 0), stop=(dc == NDC - 1))
                    if fill:
                        # keep the PE HAM window busy while the wire streams
                        nc.tensor.matmul(warm[:], ones[:], ones2[:],
                                         start=True, stop=True)
                for j in range(NJ):
                    nc.any.tensor_copy(dst[:, j * CH:(j + 1) * CH], accs[j][:])

            # --- phase 1: k and q projections ---
            load_w("wk", wk_d)
            project(kt_d, "wk", kT[:], fill=True)
            load_w("wq", wq_d)
            nc.scalar.dma_start(maskt[:], mask_d[:])
            project(qt_d, "wq", qT[:], fill=True)

            # --- phase 2: v projection interleaved with attention scores ---
            load_w("wv", wv_d)
            wo_sb = wpool.tile([DV, D], FP16)
            nc.scalar.dma_start(wo_sb[:], wo_d[:])

            rrep = {}

            def score_item(j, c):
                st = ps.tile([P, CH], FP32, tag="ps", name="st")
                nc.tensor.matmul(st[:], kT[:, c * P:(c + 1) * P],
                                 qT[:, j * CH:(j + 1) * CH],
                                 start=True, stop=True)
                et = et_all[:, ET_OFF[(j, c)]:ET_OFF[(j, c)] + CH]
                nc.scalar.activation(et, st[:],
                                     mybir.ActivationFunctionType.Exp,
                                     bias=ebias[:], scale=SCALE)
                d = c - 4 * j
                if d >= 0:   # diagonal tile: zero out k > q
                    nc.vector.tensor_mul(et, et, maskt[:, d * CH:(d + 1) * CH])
                if c == 0:
                    rrep[j] = ps.tile([P, CH], FP32, tag="ps", name=f"rrep{j}")
                nc.tensor.matmul(rrep[j][:], ones[:], et,
                                 start=(c == 0), stop=(c == 4 * j + 3))
                if c == 4 * j + 3:
                    rinv = rinv_all[:, j * CH:(j + 1) * CH]
                    nc.vector.reciprocal_approx_fast(rinv, rrep[j][:])

            vaccs = [ps.tile([P, CH], FP32, tag="ps", name=f"vacc{j}")
                     for j in range(NJ)]
            w_sb = w_sbs["wv"]
            si = 0
            for dc in range(NDC):
                # scores first in program order: they are always ready, so the
                # in-order PE queue fills DMA-wait time with them
                nxt = (len(SCORE_ITEMS) * (dc + 1)) // NDC
                while si < nxt:
                    score_item(*SCORE_ITEMS[si])
                    si += 1
                xt = xs.tile([P, L], FP16, tag="xt", name="xt")
                nc.sync.dma_start(xt[:], vt_d[dc * P:(dc + 1) * P, :])
                for j in range(NJ):
                    nc.tensor.matmul(
                        vaccs[j][:], w_sb[:, dc * P:dc * P + P],
                        xt[:, j * CH:(j + 1) * CH],
                        start=(dc == 0), stop=(dc == NDC - 1))
            while si < len(SCORE_ITEMS):
                score_item(*SCORE_ITEMS[si])
                si += 1
            for j in range(NJ):
                nc.any.tensor_copy(vT[:, j * CH:(j + 1) * CH], vaccs[j][:])

            # --- phase 3: v transposes, PV, normalize, Y, all pipelined ---
            def transposes(j):
                for c in range(4 * j, 4 * j + 4):
                    tp = ps.tile([P, P], FP16, tag="ps", name="tp")
                    nc.tensor.transpose(tp[:], vT[:, c * P:(c + 1) * P], ident[:])
                    nc.any.tensor_copy(v_nat[:, c * P:(c + 1) * P], tp[:])

            def ot_chunk(j):
                ot = ps.tile([P, CH], FP32, tag="ps", name="ot")
                for c in range(4 * j + 4):
                    nc.tensor.matmul(ot[:], v_nat[:, c * P:(c + 1) * P],
                                     et_all[:, ET_OFF[(j, c)]:ET_OFF[(j, c)] + CH],
                                     start=(c == 0), stop=(c == 4 * j + 3))
                nc.vector.tensor_mul(oT[:, j * CH:(j + 1) * CH], ot[:],
                                     rinv_all[:, j * CH:(j + 1) * CH])

            def y_chunk(j, split=False):
                for t in range(CH // P):
                    lq0 = j * CH + t * P
                    yev = ev_pool.tile([P, D], YDT, tag="yev", name="yev")
                    for dch in range(D // CH):
                        yps = ps.tile([P, CH], FP32, tag="ps", name="yps")
                        nc.tensor.matmul(yps[:], oT[:, lq0:lq0 + P],
                                         wo_sb[:, dch * CH:(dch + 1) * CH],
                                         start=True, stop=True)
                        dst = yev[:, dch * CH:(dch + 1) * CH]
                        if dch % 2 == 0:
                            nc.vector.tensor_copy(dst, yps[:])
                        else:
                            nc.scalar.copy(dst, yps[:])
                        if split:
                            nc.sync.dma_start(
                                y_d[lq0:lq0 + P, dch * CH:(dch + 1) * CH], dst)
                    if not split:
                        nc.sync.dma_start(y_d[lq0:lq0 + P, :], yev[:])

            # software pipeline: Y(j-1) hides behind OT(j)/transposes(j+1)
            transposes(0)
            ot_chunk(0)
            for j in range(1, NJ):
                transposes(j)
                ot_chunk(j)
                y_chunk(j - 1)
            y_chunk(NJ - 1)
    nc.compile()
    return nc


_NC = None


def _get_nc():
    global _NC
    if _NC is None:
        _NC = _build()
    return _NC


def _pack_w(w):
    """(D, dk) fp32 -> SBUF image (128, NDC*dk): out[p, dc*dk+m] = w[dc*128+p, m]"""
    return np.ascontiguousarray(
        w.reshape(NDC, P, -1).transpose(1, 0, 2).reshape(P, -1)).astype(np.float16)


def _make_in_maps(Q, K, V, Wq, Wk, Wv, Wo):
    f16 = np.float16
    # fold GQA group sum into Wq: head = g*KV_HEADS + h
    Wq_eff = np.asarray(Wq, np.float32).reshape(D, GROUPS, KV_HEADS, DK).sum(axis=1)
    mask = np.zeros((P, NJ * CH), f16)
    for d in range(4):
        p = np.arange(P)[:, None]
        x = np.arange(CH)[None, :]
        mask[:, d * CH:(d + 1) * CH] = (128 * d + p <= x).astype(f16)
    acts = {}
    for b in range(B):
        acts[b] = {
            "qt": np.ascontiguousarray(np.asarray(Q[b], np.float32).T).astype(f16),
            "kt": np.ascontiguousarray(np.asarray(K[b], np.float32).T).astype(f16),
            "vt": np.ascontiguousarray(np.asarray(V[b], np.float32).T).astype(f16),
        }
    Wk32, Wv32 = np.asarray(Wk, np.float32), np.asarray(Wv, np.float32)
    Wo32 = np.asarray(Wo, np.float32)
    in_maps = []
    for c in range(8):
        b, h = divmod(c, KV_HEADS)
        in_maps.append({
            **acts[b],
            "wq": _pack_w(Wq_eff[:, h, :]),
            "wk": _pack_w(Wk32[:, h * DK:(h + 1) * DK]),
            "wv": _pack_w(Wv32[:, h * DV:(h + 1) * DV]),
            "wo": Wo32[h * DV:(h + 1) * DV, :].astype(f16),
            "mask": mask,
        })
    return in_maps


def _gather(results):
    Y = np.zeros((B, L, D), np.float32)
    for c in range(8):
        Y[c // KV_HEADS] += results[c]["y"].astype(np.float32)
    return Y


def kernel(Q, K, V, Wq, Wk, Wv, Wo):
    nc = _get_nc()
    in_maps = _make_in_maps(Q, K, V, Wq, Wk, Wv, Wo)
    res = bass_utils.run_bass_kernel_spmd(nc, in_maps, core_ids=list(range(8)))
    return _gather(res.results)


def _install_ntff_hook():
    """The agent image's antenv lacks axon_hooks; synthesize it so
    trace=True can reach the NTFF profiler in libaxon_pjrt.so."""
    import types
    import antenv
    if hasattr(antenv, "axon_hooks"):
        return
    mod = types.ModuleType("antenv.axon_hooks")
    _h = [None]
    mod.set_axon_ntff_profile_hook = lambda h: _h.__setitem__(0, h)
    mod.get_axon_ntff_profile_hook = lambda: _h[0]
    sys.modules["antenv.axon_hooks"] = mod
    antenv.axon_hooks = mod
    from trn_agent_boot.trn_boot import _ntff_profile_via_ctypes
    mod.set_axon_ntff_profile_hook(_ntff_profile_via_ctypes("/opt/axon/libaxon_pjrt.so"))


def kernel_traced(Q, K, V, Wq, Wk, Wv, Wo):
    """Like kernel() but profiles; returns (output, BassKernelResults)."""
    _install_ntff_hook()
    nc = _get_nc()
    in_maps = _make_in_maps(Q, K, V, Wq, Wk, Wv, Wo)
    res = bass_utils.run_bass_kernel_spmd(nc, in_maps, core_ids=list(range(8)),
                                          trace=True)
    return _gather(res.results), res
